# revision 46
# baseline (speedup 1.0000x reference)
"""Trainium2 Bass kernel for decomposed-rel-pos attention (B=4, H=W=32, DIM=768, HEADS=12).

Sharding: 48 (batch, head) pairs -> 8 cores x 6 heads (core c: batch c//2,
heads (c%2)*6 .. +6). All matmuls run in bf16 (fp8 fails the 2e-2 gate:
measured 2.5e-2+ end-to-end; bf16 lands ~4.6e-3). The softmax scale sqrt(1/8)
is folded into both wq and wk on the host; rel tables are pre-multiplied by
1/sqrt(s), so no scaling ops run on device.

Per head: qk projection (6 x 128-row chunks), S matmul with the decomposed
rel-pos bias folded in as extra contraction rows (k'|ecomb stationary,
q'|relh|relw moving), exp on the Act engine into bf16 attnT tiles, then a
FLIPPED AV: attnT[kb] slices are the stationary operand and the 65-wide
V(+ones) slice is moving -> 65 cols x 64 matmuls instead of 8192 cols
(halves AV PE time). The AV output lands query-major [128q, 65], so the
softmax denominator is a per-partition scalar: reciprocal_approx_fast +
tensor_scalar_mul normalize straight into bf16.

PSUM rules learned on hardware: `start` resets the whole bank's accumulation
buffer (committed values persist), so the four per-bank AV accumulation
groups issue `start` only on the first group (skip_group_check). GPSIMD
cannot read PSUM, so all psum->sbuf drains sit on DVE/Act.

Proj lhsT assembly: chunks 0/1 (heads 0-3) via paired xbar DMA-transposes
hidden under later ladders; chunk 2 (heads 4/5) via PE transpose + DVE copy
at the tail (no DMA round-trips on the critical path). Output is bf16
(host upcasts) with paired/single out-DMAs to cut DMA-mutex round-trips.

Pipelining: the exp ladder is the per-head clock (8x ~1.04us). Next-head
prep (qk -> casts -> relh -> relw -> copies) spans ~1.5 ladders: qk chunks
fill the exp-WAR waits at kb6/7, casts fire at kb7-post, rel stages at
kb0-2, and the next head's first two S chunks are primed at kb6/7 so the
Act chain crosses head boundaries without a gap. V projection and head-0
prep run in a prefix + ladder-0 injections while the input DMAs stream.
PSUM: S ladder 2x[128,1024] (4 banks) + AV po 2x[128,512] (2) + prep (2).
TimelineSim: 100.3us/core (baseline: 133.0us).
"""
from contextlib import ExitStack

import numpy as np
import ml_dtypes

import concourse.bass as bass
import concourse.bacc as bacc
import concourse.mybir as mybir
import concourse.tile as tile
from concourse.bass_utils import run_bass_kernel_spmd

B, H, W, DIM, HEADS = 4, 32, 32, 768, 12
HD = DIM // HEADS  # 64
N = H * W  # 1024
HPC = HEADS // 2  # heads per core = 6
NCORES = 8
F32 = mybir.dt.float32
BF16 = mybir.dt.bfloat16
EXP = mybir.ActivationFunctionType.Exp

_cache = {}


def build_program(reps=1, with_bias=False):
    nxr = DIM + (1 if with_bias else 0)
    nc = bacc.Bacc("TRN2", target_bir_lowering=False, debug=False,
                   enable_asserts=False, num_devices=NCORES)
    x_d = nc.dram_tensor("x_bf", [nxr, N], BF16, kind="ExternalInput")
    wqk_d = nc.dram_tensor("wqk", [nxr, HPC * 128], BF16, kind="ExternalInput")
    wv_d = nc.dram_tensor("wv", [nxr, HPC * 65], BF16, kind="ExternalInput")
    wp_d = nc.dram_tensor("wp", [HPC * HD, DIM], BF16, kind="ExternalInput")
    rhw_d = nc.dram_tensor("rhw", [HD, 2 * N], BF16, kind="ExternalInput")
    ec_d = nc.dram_tensor("ecomb", [64, N], BF16, kind="ExternalInput")
    id_d = nc.dram_tensor("ident", [128, 128], BF16, kind="ExternalInput")
    out_d = nc.dram_tensor("out_part", [N, DIM], BF16, kind="ExternalOutput")

    with ExitStack() as ctx:
        tc = ctx.enter_context(tile.TileContext(nc))
        _body(nc, tc, ctx, x_d, wqk_d, wv_d, wp_d, rhw_d, ec_d, id_d,
              out_d, reps, with_bias)
    nc.compile()
    return nc


def _body(nc, tc, ctx, x_d, wqk_d, wv_d, wp_d, rhw_d, ec_d, id_d,
          out_d, reps, with_bias):
    nxc = 7 if with_bias else 6  # x chunks (last is the ones row)
    nxr = DIM + (1 if with_bias else 0)
    persist = ctx.enter_context(tc.tile_pool(name="persist", bufs=1))
    attn = ctx.enter_context(tc.tile_pool(name="attn", bufs=1))
    small = ctx.enter_context(tc.tile_pool(name="small", bufs=4))
    outp = ctx.enter_context(tc.tile_pool(name="outp", bufs=4))
    ps_s = ctx.enter_context(tc.tile_pool(name="ps_s", bufs=2, space="PSUM"))
    ps_av = ctx.enter_context(tc.tile_pool(name="ps_av", bufs=1, space="PSUM"))
    ps_p = ctx.enter_context(tc.tile_pool(name="ps_p", bufs=1, space="PSUM"))

    def ptile(p, f, dt, tag):
        return persist.tile([p, f], dt, tag=tag, name=tag)

    x_sb = [ptile(128 if i < 6 else 1, N, BF16, f"x{i}") for i in range(nxc)]
    wqk_sb = [ptile(128 if i < 6 else 1, HPC * 128, BF16, f"wqk{i}") for i in range(nxc)]
    wv_all = ptile(128, 6 * HPC * 65, BF16, "wv_all")
    wv_sb = [wv_all[:, i * HPC * 65:(i + 1) * HPC * 65] for i in range(6)]
    if with_bias:
        wv_sb.append(ptile(1, HPC * 65, BF16, "wv6"))
    wp_all = ptile(128, 3 * DIM, BF16, "wp_all")
    wp_sb = [wp_all[:, t * DIM:(t + 1) * DIM] for t in range(3)]
    rhw_sb = ptile(HD, 2 * N, BF16, "rhw")
    # per-head operand tiles, rotation depth 3 (prep pipelines ~1.5 heads ahead)
    NB = 3
    lcs = [ptile(128, N, BF16, f"lc{i}") for i in range(NB)]   # q' | relh | relw
    rcs = [ptile(128, N, BF16, f"rc{i}") for i in range(NB)]   # k' | ecomb
    v_sb = [ptile(128, HPC * 65, BF16, f"v{m}") for m in range(8)]
    aoq = [ptile(128, HPC * HD, BF16, f"ao{q}") for q in range(8)]  # [q, c] normalized
    plhs_all = ptile(128, 3 * N, BF16, "plhs")  # proj lhsT [c, (t, q)]
    plhs = [plhs_all[:, t * N:(t + 1) * N] for t in range(3)]
    ident_sb = ptile(128, 128, BF16, "ident")

    # ---- input loads: x on SP ring, weights on Act ring, in consumption order
    for c in range(nxc):
        rs = slice(c * 128, min(nxr, (c + 1) * 128))
        nc.sync.dma_start(x_sb[c][:], x_d[rs, :])
        nc.scalar.dma_start(wqk_sb[c][:], wqk_d[rs, :])
    nc.scalar.dma_start(
        wv_all[:].rearrange("p (c f) -> p c f", c=6),
        wv_d[0:DIM, :].rearrange("(c p) f -> p c f", c=6))
    nc.sync.dma_start(rhw_sb[:], rhw_d[:])
    for i in range(NB):
        nc.sync.dma_start(rcs[i][64:128, :], ec_d[:])
    nc.sync.dma_start(ident_sb[:], id_d[:])
    if with_bias:
        nc.scalar.dma_start(wv_sb[6][:], wv_d[DIM:DIM + 1, :])
    nc.sync.dma_start(
        wp_all[:].rearrange("p (t f) -> p t f", t=3),
        wp_d[:].rearrange("(t p) f -> p t f", t=3))

    def ecopy(eng, out, in_):
        if eng is nc.scalar:
            eng.copy(out, in_)
        else:
            eng.tensor_copy(out, in_)

    # ---- phase builders ----
    def qk_mm(h, cs):
        """qk projection for head h, chunk subset cs; psum tag 'prep'."""
        if ("pqk", h) not in state:
            state[("pqk", h)] = ps_p.tile([128, N], F32, tag="prep", name="pqk")
        pqk = state[("pqk", h)]
        for c in cs:
            for half in (0, 1):
                sl = slice(half * 512, half * 512 + 512)
                nc.tensor.matmul(pqk[:, sl], wqk_sb[c][:, h * 128:(h + 1) * 128],
                                 x_sb[c][:, sl], start=(c == 0), stop=(c == nxc - 1))
        return pqk

    def casts(h, part="qk", eng=None):
        eng = eng or nc.vector
        lc, rc = lcs[h % NB], rcs[h % NB]
        if "q" in part:
            ecopy(eng, lc[0:64, :], state[("pqk", h)][0:64, :])
        if "k" in part:
            ecopy(eng, rc[0:64, :], state[("pqk", h)][64:128, :])
            state.pop(("pqk", h))

    def rel_h(h, eng=None):
        lc = lcs[h % NB]
        pr = ps_p.tile([128, N], F32, tag="prep", name="pr")
        state[("pr", h)] = pr
        for qh in range(32):
            sl = slice(qh * 32, qh * 32 + 32)
            nc.tensor.matmul(pr[0:32, sl], rhw_sb[:, sl], lc[0:64, sl],
                             start=True, stop=True)
        ecopy(eng or nc.vector, lc[64:96, :], pr[0:32, :])

    def rel_w(h, eng=None):
        pr = state.pop(("pr", h))
        lc = lcs[h % NB]
        q3 = lc[0:64, :].rearrange("p (a b) -> p b a", b=32)  # [64, qw, qh]
        for qw in range(32):
            sl = slice(qw * 32, qw * 32 + 32)
            nc.tensor.matmul(pr[32:64, sl], rhw_sb[:, N + qw * 32:N + qw * 32 + 32], q3[:, qw, :],
                             start=True, stop=True)
        prw_v = pr[32:64, :].rearrange("p (a b) -> p b a", b=32)  # [32, qh, qw]
        ecopy(eng or nc.vector, lc[96:128, :], prw_v)

    def v_proj(m, eng=None):
        pv = ps_s.tile([128, N], F32, tag="s", name="pv")
        for c in range(nxc):
            nc.tensor.matmul(pv[:, 0:HPC * 65], x_sb[c][:, m * 128:(m + 1) * 128],
                             wv_sb[c][:], start=(c == 0), stop=(c == nxc - 1))
        ecopy(eng or nc.vector, v_sb[m][:], pv[:, 0:HPC * 65])
        if not with_bias:
            ones = v_sb[m][:].rearrange("p (h c) -> p h c", c=65)
            nc.gpsimd.memset(ones[:, :, 64:65], 1.0)

    state = {}

    def S_mm(h, kb):
        """One S matmul pair for head h, key chunk kb; psum tag 's'."""
        lc, rc = lcs[h % NB], rcs[h % NB]
        ps = ps_s.tile([128, N], F32, tag="s", name="s_ps")
        for half in (0, 1):
            sl = slice(half * 512, half * 512 + 512)
            nc.tensor.matmul(ps[:, sl], rc[:, kb * 128:(kb + 1) * 128],
                             lc[:, sl], start=True, stop=True)
        state[("s", h, kb)] = ps

    def ladder(h, inject):
        atn = [attn.tile([128, N], BF16, tag=f"at{kb}", name=f"at{kb}") for kb in range(8)]
        po = [ps_av.tile([128, 512], F32, tag=f"po{i}", name=f"po{i}") for i in range(2)]

        def AV(kb):
            for q in range(8):
                c0 = (q % 4) * 128
                nc.tensor.matmul(po[q // 4][:, c0:c0 + 65],
                                 atn[kb][:, q * 128:(q + 1) * 128],
                                 v_sb[kb][:, h * 65:(h + 1) * 65],
                                 start=(kb == 0 and q % 4 == 0), stop=(kb == 7),
                                 skip_group_check=True)

        def norm(q):
            c0 = (q % 4) * 128
            rcp = small.tile([128, 1], F32, tag="rcp", name="rcp")
            nc.vector.reciprocal_approx_fast(out=rcp[:], in_=po[q // 4][:, c0 + 64:c0 + 65])
            if h == HPC - 1:  # Act is free after the last exp: halve the chain
                nc.scalar.activation(aoq[q][:, h * HD:(h + 1) * HD],
                                     po[q // 4][:, c0:c0 + 64],
                                     mybir.ActivationFunctionType.Copy,
                                     scale=rcp[:])
            else:
                nc.vector.tensor_scalar_mul(aoq[q][:, h * HD:(h + 1) * HD],
                                            po[q // 4][:, c0:c0 + 64], rcp[:])

        if ("s", h, 0) not in state:  # not primed by the previous ladder
            S_mm(h, 0)
            S_mm(h, 1)
        for kb in range(8):
            ps_kb = state.pop(("s", h, kb))
            if kb < 2:
                # split: frees each psum half for S(h,kb+2) one half earlier
                nc.scalar.activation(atn[kb][:, 0:512], ps_kb[:, 0:512], EXP)
                nc.scalar.activation(atn[kb][:, 512:1024], ps_kb[:, 512:1024], EXP)
            else:
                nc.scalar.activation(atn[kb][:], ps_kb[:], EXP)
            inject(kb, "pre")  # exp-independent PE filler ahead of the S'-WAR
            if kb + 2 < 8:
                S_mm(h, kb + 2)
            elif h + 1 < HPC:
                S_mm(h + 1, kb - 6)  # prime next head's first two S chunks
            if kb < 7:
                AV(kb)
            else:
                pl3 = plhs_all[:].rearrange("p (t q) -> p t q", t=3)
                # all AV7 matmuls BEFORE any norm: interleaving creates a
                # serial cascade through coarse po-tile WARs
                for q in range(8):
                    c0 = (q % 4) * 128
                    nc.tensor.matmul(po[q // 4][:, c0:c0 + 65],
                                     atn[7][:, q * 128:(q + 1) * 128],
                                     v_sb[7][:, h * 65:(h + 1) * 65],
                                     start=False, stop=True,
                                     skip_group_check=True)
                for q in range(8):
                    norm(q)
                    if h == 3:  # chunks 0+1 ready: one paired xbar transpose
                        nc.sync.dma_start_transpose(
                            pl3[:, 0:2, q * 128:(q + 1) * 128],
                            aoq[q][:, 0:256])
                if h == HPC - 1:
                    # chunk 2 via PE transpose + DVE copy: no DMA round-trips
                    tps = ps_av.tile([128, 1024], BF16, tag="po0", name="tps")
                    for q in range(8):
                        nc.tensor.transpose(tps[:, q * 128:(q + 1) * 128],
                                            aoq[q][:, 256:384], ident_sb[:])
                        nc.vector.tensor_copy(
                            plhs_all[:, 2 * N + q * 128:2 * N + (q + 1) * 128],
                            tps[:, q * 128:(q + 1) * 128])
            inject(kb, "post")

    # ---- main schedule ----
    def proj_mm(m, pool, ts, stop):
        if ("pp", m) not in state:
            state[("pp", m)] = pool.tile([128, N], F32,
                                         tag="s" if pool is ps_s else "prep",
                                         name="pp")
        pp = state[("pp", m)]
        for t in ts:
            for n0, nw in ((0, 512), (512, 256)):
                nc.tensor.matmul(pp[:, n0:n0 + nw],
                                 plhs_all[:, t * N + m * 128:t * N + (m + 1) * 128],
                                 wp_sb[t][:, n0:n0 + nw],
                                 start=(t == 0), stop=(stop and t == ts[-1]))
        return pp

    for _rep in range(reps):
        # prefix: head-0 prep + early V; Act idles here regardless
        qk_mm(0, range(nxc))
        casts(0, "q")
        casts(0, "k", eng=nc.scalar)
        # head-0 rel: psum in the AV banks (free in the prefix) so the rel
        # matmuls don't WAR-wait on the k-cast via the shared prep tile
        lc0 = lcs[0]
        prh = ps_av.tile([128, 512], F32, tag="po0", name="prh0")
        prw = ps_av.tile([128, 512], F32, tag="po1", name="prw0")
        for qh in range(32):
            nc.tensor.matmul(prh[(qh // 16) * 32:(qh // 16) * 32 + 32,
                                 (qh % 16) * 32:(qh % 16) * 32 + 32],
                             rhw_sb[:, qh * 32:qh * 32 + 32],
                             lc0[0:64, qh * 32:qh * 32 + 32],
                             start=True, stop=True)
        nc.vector.tensor_copy(lc0[64:96, 0:512], prh[0:32, 0:512])
        nc.vector.tensor_copy(lc0[64:96, 512:1024], prh[32:64, 0:512])
        q3 = lc0[0:64, :].rearrange("p (a b) -> p b a", b=32)  # [64, qw, qh]
        for qw in range(32):
            nc.tensor.matmul(prw[(qw // 16) * 32:(qw // 16) * 32 + 32,
                                 (qw % 16) * 32:(qw % 16) * 32 + 32],
                             rhw_sb[:, N + qw * 32:N + qw * 32 + 32],
                             q3[:, qw, :], start=True, stop=True)
        dst_w = lc0[96:128, :].rearrange("p (a b) -> p a b", a=32)  # [32,qh,qw]
        nc.vector.tensor_copy(dst_w[:, :, 0:16],
                              prw[0:32, 0:512].rearrange("p (a b) -> p b a", b=32))
        nc.vector.tensor_copy(dst_w[:, :, 16:32],
                              prw[32:64, 0:512].rearrange("p (a b) -> p b a", b=32))
        if _rep == 0:
            v_proj(0, eng=nc.scalar)
            v_proj(1, eng=nc.scalar)
        S_mm(0, 0)
        S_mm(0, 1)
        if _rep == 0:
            v_proj(2, eng=nc.scalar)
        qk_mm(1, range(nxc))

        for h in range(HPC):
            def inject(kb, phase, h=h):
                if phase == "pre":
                    # qk(h+2) fills the exp-WAR wait at kb6/7
                    if kb == 6 and h < HPC - 2:
                        qk_mm(h + 2, range(0, 3))
                    elif kb == 7 and h < HPC - 2:
                        qk_mm(h + 2, range(3, nxc))
                    return
                if _rep == 0 and h == 0 and 0 <= kb <= 4:
                    v_proj(kb + 3)
                if kb == 0 and h + 1 < HPC and ("pqk", h + 1) in state:
                    casts(h + 1)
                elif kb == 1 and h + 1 < HPC:
                    rel_h(h + 1)
                elif kb == 2 and h + 1 < HPC:
                    rel_w(h + 1)
                elif kb == 7 and h + 2 < HPC:
                    casts(h + 2)
                elif kb == 5 and h == HPC - 2:
                    proj_mm(0, ps_p, (0, 1), stop=False)
                elif kb == 6 and h == HPC - 1:
                    proj_mm(1, ps_s, (0, 1), stop=False)
                elif kb == 7 and h == HPC - 1:
                    proj_mm(2, ps_s, (0, 1), stop=False)
            ladder(h, inject)

    # ---- projection tail (m0-m2 prefetched t0/t1; t2 gated on transposes;
    #      out-DMAs paired to halve DMA-mutex round-trips) ----
    osb2 = [ptile(128, 2 * DIM, BF16, f"osb{j}") for j in range(4)]
    for m in range(8):
        if m < 3:
            pp = proj_mm(m, None, (2,), stop=True)
        else:
            pp = proj_mm(m, ps_s if m % 3 < 2 else ps_p, (0, 1, 2), stop=True)
        state.pop(("pp", m))
        dst = osb2[m // 2][:, (m % 2) * DIM:(m % 2) * DIM + DIM]
        if m == 7:
            nc.scalar.copy(dst[:, 0:DIM // 2], pp[:, 0:DIM // 2])
            nc.vector.tensor_copy(dst[:, DIM // 2:DIM], pp[:, DIM // 2:DIM])
        elif m % 2 == 0:
            nc.scalar.copy(dst, pp[:, 0:DIM])
        else:
            nc.vector.tensor_copy(dst, pp[:, 0:DIM])
        if m in (1, 3, 5):
            src_ap = osb2[m // 2][:].rearrange("p (j d) -> p j d", j=2)
            dst_ap = out_d[(m - 1) * 128:(m + 1) * 128, :].rearrange(
                "(j p) d -> p j d", j=2)
            nc.sync.dma_start(dst_ap, src_ap)
        elif m == 6:
            nc.sync.dma_start(out_d[6 * 128:7 * 128, :], dst)
        elif m == 7:
            nc.sync.dma_start(out_d[7 * 128:8 * 128, :], dst)


def _host_prep(x, qkv_w, qkv_b, proj_w, proj_b, rel_pos_h, rel_pos_w, with_bias):
    BF = ml_dtypes.bfloat16
    sq = float(HD ** -0.25)  # sqrt(softmax scale), folded into wq and wk
    idx_h = np.arange(H)[:, None] - np.arange(H)[None, :] + (H - 1)
    idx_w = np.arange(W)[:, None] - np.arange(W)[None, :] + (W - 1)
    Rh = rel_pos_h[idx_h] / sq  # [qh, kh, c]
    Rw = rel_pos_w[idx_w] / sq
    rhw = np.concatenate([
        Rh.transpose(2, 0, 1).reshape(HD, H * H),
        Rw.transpose(2, 0, 1).reshape(HD, W * W)], 1).astype(BF)
    kt = np.arange(N)
    ec = np.zeros((64, N), np.float32)
    ec[:32] = (np.arange(32)[:, None] == (kt // 32)[None, :])
    ec[32:] = (np.arange(32)[:, None] == (kt % 32)[None, :])
    ec = ec.astype(BF)

    nxr = DIM + (1 if with_bias else 0)
    in_maps = []
    for core in range(NCORES):
        b = core // 2
        h0 = (core % 2) * HPC
        xT = np.empty((nxr, N), np.float32)
        xT[:DIM] = x[b].reshape(N, DIM).T
        if with_bias:
            xT[DIM] = 1.0
        wqk = np.zeros((nxr, HPC * 128), np.float32)
        wv = np.zeros((nxr, HPC * 65), np.float32)
        wpm = np.zeros((HPC * HD, DIM), np.float32)
        for h in range(HPC):
            g = h0 + h
            wqk[:DIM, h * 128:h * 128 + 64] = qkv_w[g * HD:(g + 1) * HD].T * sq
            wqk[:DIM, h * 128 + 64:h * 128 + 128] = qkv_w[DIM + g * HD:DIM + (g + 1) * HD].T * sq
            wv[:DIM, h * 65:h * 65 + 64] = qkv_w[2 * DIM + g * HD:2 * DIM + (g + 1) * HD].T
            if with_bias:
                wqk[DIM, h * 128:h * 128 + 64] = qkv_b[g * HD:(g + 1) * HD] * sq
                wqk[DIM, h * 128 + 64:h * 128 + 128] = qkv_b[DIM + g * HD:DIM + (g + 1) * HD] * sq
                wv[DIM, h * 65:h * 65 + 64] = qkv_b[2 * DIM + g * HD:2 * DIM + (g + 1) * HD]
                wv[DIM, h * 65 + 64] = 1.0
            wpm[h * HD:(h + 1) * HD, :] = proj_w[:, g * HD:(g + 1) * HD].T
        in_maps.append({
            "x_bf": xT.astype(BF), "wqk": wqk.astype(BF), "wv": wv.astype(BF),
            "wp": wpm.astype(BF), "rhw": rhw, "ecomb": ec,
            "ident": np.eye(128, dtype=np.float32).astype(BF),
        })
    return in_maps


def kernel(x, qkv_w, qkv_b, proj_w, proj_b, rel_pos_h, rel_pos_w, _trace=False):
    x = np.asarray(x, np.float32)
    qkv_w = np.asarray(qkv_w, np.float32)
    qkv_b = np.asarray(qkv_b, np.float32)
    proj_w = np.asarray(proj_w, np.float32)
    proj_b = np.asarray(proj_b, np.float32)
    rel_pos_h = np.asarray(rel_pos_h, np.float32)
    rel_pos_w = np.asarray(rel_pos_w, np.float32)

    with_bias = bool(np.any(qkv_b))
    in_maps = _host_prep(x, qkv_w, qkv_b, proj_w, proj_b, rel_pos_h, rel_pos_w,
                         with_bias)
    key = ("nc", with_bias)
    if key not in _cache:
        _cache[key] = build_program(with_bias=with_bias)
    nc = _cache[key]
    res = run_bass_kernel_spmd(nc, in_maps, core_ids=list(range(NCORES)),
                               trace=_trace)
    parts = [np.asarray(r["out_part"], np.float32) for r in res.results]
    out = np.zeros((B, N, DIM), np.float32)
    for b in range(B):
        out[b] = parts[2 * b] + parts[2 * b + 1] + proj_b
    if _trace:
        kernel.last_results = res
    return out.reshape(B, H, W, DIM)
